# revision 8
# baseline (speedup 1.0000x reference)
"""Trainium2 Bass kernel: 3x3 valid 2D cross-correlation on an 8192x8192 f32 image.

Strategy (8 NeuronCores, pure spatial/data parallel, fp16 I/O):
  - The rel-err budget (2e-2) admits fp16 end-to-end (~5e-4 actual), which
    halves HBM traffic vs f32: per core ~17.1 MB read + ~16.8 MB write at
    ~360 GB/s/core => ~94 us DMA floor (the kernel is HBM-bandwidth-bound).
  - Column-shard on the host: core i receives the fp16 input column slab
    [8192, 1026] = cols [1024*i, 1024*i + 1026) (2-col halo; right edge
    zero-padded, garbage output cols discarded on host).
  - Per core, 65 identical row tiles: [128 in rows -> 126 out rows] x 1024
    out cols (65*126 = 8190 exactly; tile t reads rows [126t, 126t+128)).
    Per tile, 2 PSUM chunks of 512 cols; per chunk 3 TensorEngine matmuls
    (fp16 operands, 1 col/cycle) accumulate:
        out[y, c] = sum_dx (M_dx.T @ X)[y, c+dx]
    where M_dx[k, y] = w[k-y, dx] is a 3-diagonal band matrix built on the
    host. PE streams 3*1024 cols/tile => ~83 us total, under the DMA floor.
  - SP ring does x loads (263 KB/tile), Pool ring (SWDGE) does y stores
    (258 KB/tile), ScalarE copies even chunks PSUM->SBUF (f32->fp16),
    VectorE odd chunks. 8 PSUM banks = 4 tiles in flight.
"""

import numpy as np

import concourse.bass as bass
import concourse.mybir as mybir
from concourse.bass_utils import run_bass_kernel_spmd

H = W = 8192
KH = KW = 3
N_CORES = 8
OUT_H = H - KH + 1  # 8190
OUT_W = W - KW + 1  # 8190

COLS_PER_CORE = 1024          # output cols per core (core 7: keep 1022)
IN_COLS = COLS_PER_CORE + KW - 1  # 1026
TILE_OUT = 126                # output rows per 128-partition input tile
TILE_IN = TILE_OUT + KH - 1   # 128
N_TILES = OUT_H // TILE_OUT   # 65 (exact)
CHUNK = 512                   # PSUM bank width (fp32)
N_CHUNKS = COLS_PER_CORE // CHUNK  # 2
XBUFS = 12
OBUFS = 8

_NC_CACHE = {}


def _build_program():
    nc = bass.Bass("TRN2", target_bir_lowering=False, debug=False)
    x = nc.declare_dram_parameter(
        "x", [H, IN_COLS], mybir.dt.float16, isOutput=False
    )
    m = nc.declare_dram_parameter(
        "m", [128, KW * TILE_OUT], mybir.dt.float16, isOutput=False
    )
    y = nc.declare_dram_parameter(
        "y", [OUT_H, COLS_PER_CORE], mybir.dt.float16, isOutput=True
    )

    xb = [nc.alloc_sbuf_tensor(f"xb{i}", [128, IN_COLS], mybir.dt.float16).ap()
          for i in range(XBUFS)]
    ob = [nc.alloc_sbuf_tensor(f"ob{i}", [128, COLS_PER_CORE],
                               mybir.dt.float16).ap()
          for i in range(OBUFS)]
    mt = nc.alloc_sbuf_tensor("mt", [128, KW * TILE_OUT], mybir.dt.float16).ap()
    pb = [nc.alloc_psum_tensor(f"pb{i}", [128, CHUNK], mybir.dt.float32).ap()
          for i in range(8)]

    sm = nc.alloc_semaphore("sm")
    # Per-slot DMA-completion semaphores: DMA instructions on real HW can
    # complete out of order across the 16 engines, so a single counting
    # semaphore cannot prove that a *specific* transfer finished.
    sxs = [nc.alloc_semaphore(f"sxs{s}") for s in range(XBUFS)]
    sos = [nc.alloc_semaphore(f"sos{o}") for o in range(OBUFS)]
    s_mm = nc.alloc_semaphore("s_mm")
    s_cpA = nc.alloc_semaphore("s_cpA")
    s_cpD = nc.alloc_semaphore("s_cpD")

    with nc.Block() as block:

        # SP ring: all x loads, plus odd-tile y stores (DVE has no DMA ring;
        # ACT takes the even-tile stores). Store t-1 is emitted after load t
        # so readiness stays monotone along the ring (load t unblocks at
        # "PE consumed tile t-XBUFS", store t-1 at "copies of t-1 done").
        @block.sync
        def _(sync):
            sync.dma_start(out=mt, in_=m[:]).then_inc(sm, 16)
            for t in range(N_TILES):
                if t >= XBUFS:
                    # x slot reuse: tile in this slot fully consumed by PE
                    sync.wait_ge(s_mm, N_CHUNKS * (t - XBUFS + 1))
                r0 = t * TILE_OUT
                sync.dma_start(
                    out=xb[t % XBUFS][:TILE_IN], in_=x[r0:r0 + TILE_IN, :]
                ).then_inc(sxs[t % XBUFS], 16)
                ts = t - 1
                if ts >= 0 and ts % 2 == 1:
                    sync.wait_ge(s_cpA, ts + 1)
                    sync.wait_ge(s_cpD, ts + 1)
                    r0 = ts * TILE_OUT
                    sync.dma_start(
                        out=y[r0:r0 + TILE_OUT, :],
                        in_=ob[ts % OBUFS][:TILE_OUT, :],
                    ).then_inc(sos[ts % OBUFS], 16)
            for o in range(OBUFS):
                n_stores = (N_TILES - o + OBUFS - 1) // OBUFS
                sync.wait_ge(sos[o], 16 * n_stores)

        @block.tensor
        def _(tensor):
            tensor.wait_ge(sm, 16)
            for t in range(N_TILES):
                tensor.wait_ge(sxs[t % XBUFS], 16 * (t // XBUFS + 1))
                for ci in range(N_CHUNKS):
                    g = t * N_CHUNKS + ci
                    b = g % 8
                    if g >= 8:
                        # PSUM bank b free once chunk g-8's copy retired;
                        # chunk parity == bank parity, so g-8 has parity ci
                        if ci == 0:
                            tensor.wait_ge(s_cpA, t - 3)
                        else:
                            tensor.wait_ge(s_cpD, t - 3)
                    c0 = ci * CHUNK
                    for dx in range(KW):
                        ins = nc.tensor.matmul(
                            pb[b][:TILE_OUT, :CHUNK],
                            mt[:TILE_IN, dx * TILE_OUT:(dx + 1) * TILE_OUT],
                            xb[t % XBUFS][:TILE_IN, c0 + dx:c0 + dx + CHUNK],
                            start=(dx == 0),
                            stop=(dx == KW - 1),
                        )
                        if dx == KW - 1:
                            ins.then_inc(s_mm, 1)

        @block.scalar
        def _(scalar):
            for t in range(N_TILES):
                if t >= OBUFS:
                    scalar.wait_ge(sos[t % OBUFS], 16 * (t // OBUFS))
                scalar.wait_ge(s_mm, N_CHUNKS * t + 1)
                nc.scalar.copy(
                    out=ob[t % OBUFS][:TILE_OUT, 0:CHUNK],
                    in_=pb[(N_CHUNKS * t) % 8][:TILE_OUT, :CHUNK],
                ).then_inc(s_cpA, 1)
                if t % 2 == 0:
                    scalar.wait_ge(s_cpD, t + 1)
                    r0 = t * TILE_OUT
                    scalar.dma_start(
                        out=y[r0:r0 + TILE_OUT, :],
                        in_=ob[t % OBUFS][:TILE_OUT, :],
                    ).then_inc(sos[t % OBUFS], 16)

        @block.vector
        def _(vector):
            for t in range(N_TILES):
                if t >= OBUFS:
                    vector.wait_ge(sos[t % OBUFS], 16 * (t // OBUFS))
                vector.wait_ge(s_mm, N_CHUNKS * t + 2)
                nc.vector.tensor_copy(
                    out=ob[t % OBUFS][:TILE_OUT, CHUNK:COLS_PER_CORE],
                    in_=pb[(N_CHUNKS * t + 1) % 8][:TILE_OUT, :CHUNK],
                ).then_inc(s_cpD, 1)

    return nc


def _get_program():
    if "nc" not in _NC_CACHE:
        _NC_CACHE["nc"] = _build_program()
    return _NC_CACHE["nc"]


def _band_matrices(weight: np.ndarray) -> np.ndarray:
    """m[k, dx*126 + y] = w[k-y, dx] for 0 <= k-y < 3."""
    mm = np.zeros((128, KW * TILE_OUT), dtype=np.float32)
    for dx in range(KW):
        for dy in range(KH):
            ys = np.arange(TILE_OUT)
            mm[ys + dy, dx * TILE_OUT + ys] = weight[dy, dx]
    return mm.astype(np.float16)


def _in_maps(x, weight):
    mmat = _band_matrices(weight)
    xh = np.asarray(x, dtype=np.float32).astype(np.float16)
    maps = []
    for i in range(N_CORES):
        c0 = i * COLS_PER_CORE
        c1 = min(c0 + IN_COLS, W)
        slab = np.zeros((H, IN_COLS), dtype=np.float16)
        slab[:, : c1 - c0] = xh[:, c0:c1]
        maps.append({"x": np.ascontiguousarray(slab), "m": mmat})
    return maps


def kernel(x: np.ndarray, weight: np.ndarray) -> np.ndarray:
    x = np.asarray(x, dtype=np.float32)
    weight = np.asarray(weight, dtype=np.float32)
    assert x.shape == (H, W) and weight.shape == (KH, KW)

    nc = _get_program()
    res = run_bass_kernel_spmd(nc, _in_maps(x, weight),
                               core_ids=list(range(N_CORES)))

    out = np.empty((OUT_H, OUT_W), dtype=np.float32)
    for i in range(N_CORES):
        c0 = i * COLS_PER_CORE
        keep = min(COLS_PER_CORE, OUT_W - c0)
        out[:, c0:c0 + keep] = res.results[i]["y"][:, :keep].astype(np.float32)
    return out


# revision 9
# speedup vs baseline: 2.5731x; 2.5731x over previous
"""Trainium2 Bass kernel: 3x3 valid 2D cross-correlation on an 8192x8192 f32 image.

Strategy (8 NeuronCores, pure spatial/data parallel, fp16 I/O):
  - The rel-err budget (2e-2) admits fp16 end-to-end (~5e-4 actual), which
    halves HBM traffic vs f32: per core ~17.1 MB read + ~16.8 MB write at
    ~360 GB/s/core => ~94 us DMA floor (the kernel is HBM-bandwidth-bound).
  - Column-shard on the host: core i receives the fp16 input column slab
    [8192, 1026] = cols [1024*i, 1024*i + 1026) (2-col halo; right edge
    zero-padded, garbage output cols discarded on host).
  - Per core, 65 identical row tiles: [128 in rows -> 126 out rows] x 1024
    out cols (65*126 = 8190 exactly; tile t reads rows [126t, 126t+128)).
    Per tile, 2 PSUM chunks of 512 cols; per chunk 3 TensorEngine matmuls
    (fp16 operands, 1 col/cycle) accumulate:
        out[y, c] = sum_dx (M_dx.T @ X)[y, c+dx]
    where M_dx[k, y] = w[k-y, dx] is a 3-diagonal band matrix built on the
    host. PE streams 3*1024 cols/tile => ~83 us total, under the DMA floor.
  - SP ring does x loads (263 KB/tile), Pool ring (SWDGE) does y stores
    (258 KB/tile), ScalarE copies even chunks PSUM->SBUF (f32->fp16),
    VectorE odd chunks. 8 PSUM banks = 4 tiles in flight.
"""

import numpy as np

import concourse.bass as bass
import concourse.mybir as mybir
from concourse.bass_utils import run_bass_kernel_spmd

H = W = 8192
KH = KW = 3
N_CORES = 8
OUT_H = H - KH + 1  # 8190
OUT_W = W - KW + 1  # 8190

COLS_PER_CORE = 1024          # output cols per core (core 7: keep 1022)
IN_COLS = COLS_PER_CORE + KW - 1  # 1026
TILE_OUT = 126                # output rows per 128-partition input tile
TILE_IN = TILE_OUT + KH - 1   # 128
N_TILES = OUT_H // TILE_OUT   # 65 (exact)
CHUNK = 512                   # PSUM bank width (fp32)
N_CHUNKS = COLS_PER_CORE // CHUNK  # 2
XBUFS = 12
OBUFS = 8

_NC_CACHE = {}


def _build_program():
    nc = bass.Bass("TRN2", target_bir_lowering=False, debug=False)
    x = nc.declare_dram_parameter(
        "x", [H, IN_COLS], mybir.dt.float16, isOutput=False
    )
    m = nc.declare_dram_parameter(
        "m", [128, KW * TILE_OUT], mybir.dt.float16, isOutput=False
    )
    y = nc.declare_dram_parameter(
        "y", [OUT_H, COLS_PER_CORE], mybir.dt.float16, isOutput=True
    )

    xb = [nc.alloc_sbuf_tensor(f"xb{i}", [128, IN_COLS], mybir.dt.float16).ap()
          for i in range(XBUFS)]
    ob = [nc.alloc_sbuf_tensor(f"ob{i}", [128, COLS_PER_CORE],
                               mybir.dt.float16).ap()
          for i in range(OBUFS)]
    mt = nc.alloc_sbuf_tensor("mt", [128, KW * TILE_OUT], mybir.dt.float16).ap()
    pb = [nc.alloc_psum_tensor(f"pb{i}", [128, CHUNK], mybir.dt.float32).ap()
          for i in range(8)]

    sm = nc.alloc_semaphore("sm")
    # Per-slot DMA-completion semaphores: DMA instructions on real HW can
    # complete out of order across the 16 engines, so a single counting
    # semaphore cannot prove that a *specific* transfer finished.
    sxs = [nc.alloc_semaphore(f"sxs{s}") for s in range(XBUFS)]
    sos = [nc.alloc_semaphore(f"sos{o}") for o in range(OBUFS)]
    s_mm = nc.alloc_semaphore("s_mm")
    s_cpA = nc.alloc_semaphore("s_cpA")
    s_cpD = nc.alloc_semaphore("s_cpD")

    with nc.Block() as block:

        # SP ring: all x loads, plus odd-tile y stores (DVE has no DMA ring;
        # ACT takes the even-tile stores). Store t-XBUFS is emitted after
        # load t so ring-order readiness stays monotone: load t unblocks at
        # "PE consumed tile t-XBUFS", store t-XBUFS ~0.5us later ("copies of
        # t-XBUFS done") — interleaving with a shorter lag would head-of-line
        # block the loads and collapse the prefetch pipeline.
        def _store(eng, ts):
            eng.wait_ge(s_cpA, ts + 1)
            eng.wait_ge(s_cpD, ts + 1)
            r0 = ts * TILE_OUT
            eng.dma_start(
                out=y[r0:r0 + TILE_OUT, :],
                in_=ob[ts % OBUFS][:TILE_OUT, :],
            ).then_inc(sos[ts % OBUFS], 16)

        @block.sync
        def _(sync):
            sync.dma_start(out=mt, in_=m[:]).then_inc(sm, 16)
            for t in range(N_TILES):
                if t >= XBUFS:
                    # x slot reuse: tile in this slot fully consumed by PE
                    sync.wait_ge(s_mm, N_CHUNKS * (t - XBUFS + 1))
                r0 = t * TILE_OUT
                sync.dma_start(
                    out=xb[t % XBUFS][:TILE_IN], in_=x[r0:r0 + TILE_IN, :]
                ).then_inc(sxs[t % XBUFS], 16)
                ts = t - XBUFS
                if ts >= 0 and ts % 2 == 1:
                    _store(sync, ts)
            for ts in range(N_TILES - XBUFS, N_TILES):
                if ts >= 0 and ts % 2 == 1:
                    _store(sync, ts)
            for o in range(OBUFS):
                n_stores = (N_TILES - o + OBUFS - 1) // OBUFS
                sync.wait_ge(sos[o], 16 * n_stores)

        @block.tensor
        def _(tensor):
            tensor.wait_ge(sm, 16)
            for t in range(N_TILES):
                tensor.wait_ge(sxs[t % XBUFS], 16 * (t // XBUFS + 1))
                for ci in range(N_CHUNKS):
                    g = t * N_CHUNKS + ci
                    b = g % 8
                    if g >= 8:
                        # PSUM bank b free once chunk g-8's copy retired;
                        # chunk parity == bank parity, so g-8 has parity ci
                        if ci == 0:
                            tensor.wait_ge(s_cpA, t - 3)
                        else:
                            tensor.wait_ge(s_cpD, t - 3)
                    c0 = ci * CHUNK
                    for dx in range(KW):
                        ins = nc.tensor.matmul(
                            pb[b][:TILE_OUT, :CHUNK],
                            mt[:TILE_IN, dx * TILE_OUT:(dx + 1) * TILE_OUT],
                            xb[t % XBUFS][:TILE_IN, c0 + dx:c0 + dx + CHUNK],
                            start=(dx == 0),
                            stop=(dx == KW - 1),
                        )
                        if dx == KW - 1:
                            ins.then_inc(s_mm, 1)

        @block.scalar
        def _(scalar):
            for t in range(N_TILES):
                if t >= OBUFS:
                    scalar.wait_ge(sos[t % OBUFS], 16 * (t // OBUFS))
                scalar.wait_ge(s_mm, N_CHUNKS * t + 1)
                nc.scalar.copy(
                    out=ob[t % OBUFS][:TILE_OUT, 0:CHUNK],
                    in_=pb[(N_CHUNKS * t) % 8][:TILE_OUT, :CHUNK],
                ).then_inc(s_cpA, 1)
                if t % 2 == 0:
                    scalar.wait_ge(s_cpD, t + 1)
                    r0 = t * TILE_OUT
                    scalar.dma_start(
                        out=y[r0:r0 + TILE_OUT, :],
                        in_=ob[t % OBUFS][:TILE_OUT, :],
                    ).then_inc(sos[t % OBUFS], 16)

        @block.vector
        def _(vector):
            for t in range(N_TILES):
                if t >= OBUFS:
                    vector.wait_ge(sos[t % OBUFS], 16 * (t // OBUFS))
                vector.wait_ge(s_mm, N_CHUNKS * t + 2)
                nc.vector.tensor_copy(
                    out=ob[t % OBUFS][:TILE_OUT, CHUNK:COLS_PER_CORE],
                    in_=pb[(N_CHUNKS * t + 1) % 8][:TILE_OUT, :CHUNK],
                ).then_inc(s_cpD, 1)

    return nc


def _get_program():
    if "nc" not in _NC_CACHE:
        _NC_CACHE["nc"] = _build_program()
    return _NC_CACHE["nc"]


def _band_matrices(weight: np.ndarray) -> np.ndarray:
    """m[k, dx*126 + y] = w[k-y, dx] for 0 <= k-y < 3."""
    mm = np.zeros((128, KW * TILE_OUT), dtype=np.float32)
    for dx in range(KW):
        for dy in range(KH):
            ys = np.arange(TILE_OUT)
            mm[ys + dy, dx * TILE_OUT + ys] = weight[dy, dx]
    return mm.astype(np.float16)


def _in_maps(x, weight):
    mmat = _band_matrices(weight)
    xh = np.asarray(x, dtype=np.float32).astype(np.float16)
    maps = []
    for i in range(N_CORES):
        c0 = i * COLS_PER_CORE
        c1 = min(c0 + IN_COLS, W)
        slab = np.zeros((H, IN_COLS), dtype=np.float16)
        slab[:, : c1 - c0] = xh[:, c0:c1]
        maps.append({"x": np.ascontiguousarray(slab), "m": mmat})
    return maps


def kernel(x: np.ndarray, weight: np.ndarray) -> np.ndarray:
    x = np.asarray(x, dtype=np.float32)
    weight = np.asarray(weight, dtype=np.float32)
    assert x.shape == (H, W) and weight.shape == (KH, KW)

    nc = _get_program()
    res = run_bass_kernel_spmd(nc, _in_maps(x, weight),
                               core_ids=list(range(N_CORES)))

    out = np.empty((OUT_H, OUT_W), dtype=np.float32)
    for i in range(N_CORES):
        c0 = i * COLS_PER_CORE
        keep = min(COLS_PER_CORE, OUT_W - c0)
        out[:, c0:c0 + keep] = res.results[i]["y"][:, :keep].astype(np.float32)
    return out


# revision 10
# speedup vs baseline: 2.5864x; 1.0052x over previous
"""Trainium2 Bass kernel: 3x3 valid 2D cross-correlation on an 8192x8192 f32 image.

Strategy (8 NeuronCores, pure spatial/data parallel, fp16 I/O):
  - The rel-err budget (2e-2) admits fp16 end-to-end (~3.5e-4 actual), which
    halves HBM traffic vs f32: per core ~17.1 MB read + ~16.8 MB write at
    ~400 GB/s/core => ~85 us DMA floor (the kernel is HBM-bandwidth-bound,
    with the TensorEngine nearly co-critical at ~92 us).
  - Column-shard on the host: core i receives the fp16 input column slab
    [8192, 1026] = cols [1024*i, 1024*i + 1026) (2-col halo; right edge
    zero-padded, garbage output cols discarded on host).
  - Per core, 65 identical row tiles: [128 in rows -> 126 out rows] x 1024
    out cols (65*126 = 8190 exactly; tile t reads rows [126t, 126t+128)).
    Per tile, 2 PSUM chunks of 512 cols; per chunk 3 TensorEngine matmuls
    (fp16 operands, 1 col/cycle) accumulate:
        out[y, c] = sum_dx (M_dx.T @ X)[y, c+dx]
    where M_dx[k, y] = w[k-y, dx] is a 3-diagonal band matrix built on the
    host from the 3x3 weight.
  - DMA-instruction issue is expensive (~1.2 us seq+HWDGE per instruction),
    so transfers are batched via custom 3-D DRAM access patterns: loads move
    LK=4 tiles per instruction (the 2-row inter-tile halo makes the read AP
    overlap, which is legal), stores move SK=2 tiles. All x loads and all
    y stores issue on the SP ring; the m load goes on the ACT ring; ACT/DVE
    only do PSUM->SBUF copies (f32->fp16, even/odd chunks).
  - DMA completions can retire out of order across the 16 DMA engines, so
    every buffer slot gets its own semaphore (a single counting semaphore
    cannot prove a *specific* transfer finished).
"""

import numpy as np

import concourse.bass as bass
import concourse.mybir as mybir
from concourse.bass_utils import run_bass_kernel_spmd

H = W = 8192
KH = KW = 3
N_CORES = 8
OUT_H = H - KH + 1  # 8190
OUT_W = W - KW + 1  # 8190

COLS_PER_CORE = 1024          # output cols per core (core 7: keep 1022)
IN_COLS = COLS_PER_CORE + KW - 1  # 1026
TILE_OUT = 126                # output rows per 128-partition input tile
TILE_IN = TILE_OUT + KH - 1   # 128
N_TILES = OUT_H // TILE_OUT   # 65 (exact)
CHUNK = 512                   # PSUM bank width (fp32)
N_CHUNKS = COLS_PER_CORE // CHUNK  # 2

LK = 4                        # tiles per load group
XG = 4                        # xb group slots (16 tiles of runway)
N_LG = (N_TILES + LK - 1) // LK  # 17 (last group 1 tile)
SK = 2                        # tiles per store batch
OG = 8                        # ob batch slots (16 tiles of cushion)
OBUFS = OG * SK               # 16 tile slots
N_SB = (N_TILES + SK - 1) // SK  # 33 (last batch 1 tile)

_NC_CACHE = {}


def _build_program():
    nc = bass.Bass("TRN2", target_bir_lowering=False, debug=False)
    x = nc.declare_dram_parameter(
        "x", [H, IN_COLS], mybir.dt.float16, isOutput=False
    )
    m = nc.declare_dram_parameter(
        "m", [128, KW * TILE_OUT], mybir.dt.float16, isOutput=False
    )
    y = nc.declare_dram_parameter(
        "y", [OUT_H, COLS_PER_CORE], mybir.dt.float16, isOutput=True
    )

    xb = nc.alloc_sbuf_tensor(
        "xb", [128, XG * LK * IN_COLS], mybir.dt.float16).ap()
    ob = nc.alloc_sbuf_tensor(
        "ob", [128, OBUFS * COLS_PER_CORE], mybir.dt.float16).ap()
    mt = nc.alloc_sbuf_tensor("mt", [128, KW * TILE_OUT], mybir.dt.float16).ap()
    pb = [nc.alloc_psum_tensor(f"pb{i}", [128, CHUNK], mybir.dt.float32).ap()
          for i in range(8)]

    sm = nc.alloc_semaphore("sm")
    sxg = [nc.alloc_semaphore(f"sxg{s}") for s in range(XG)]
    sob = [nc.alloc_semaphore(f"sob{o}") for o in range(OG)]
    s_mm = nc.alloc_semaphore("s_mm")
    s_cpA = nc.alloc_semaphore("s_cpA")
    s_cpD = nc.alloc_semaphore("s_cpD")

    def load_group_aps(g):
        nt = min(LK, N_TILES - LK * g)
        r0 = LK * g * TILE_OUT
        in_ap = x[r0:r0 + TILE_IN, :].unsqueeze(1)
        in_ap.ap = mybir.VecI64Pair(
            [[IN_COLS, TILE_IN], [TILE_OUT * IN_COLS, nt], [1, IN_COLS]]
        )
        cb = (g % XG) * LK * IN_COLS
        out_ap = xb[:TILE_IN, cb:cb + nt * IN_COLS]
        return out_ap, in_ap

    def store_batch_aps(b):
        nt = min(SK, N_TILES - SK * b)
        r0 = SK * b * TILE_OUT
        out_ap = y[r0:r0 + TILE_OUT, :].unsqueeze(1)
        out_ap.ap = mybir.VecI64Pair(
            [[COLS_PER_CORE, TILE_OUT],
             [TILE_OUT * COLS_PER_CORE, nt],
             [1, COLS_PER_CORE]]
        )
        cb = ((SK * b) % OBUFS) * COLS_PER_CORE
        in_ap = ob[:TILE_OUT, cb:cb + nt * COLS_PER_CORE]
        return out_ap, in_ap

    def emit_store(eng, b):
        nt = min(SK, N_TILES - SK * b)
        eng.wait_ge(s_cpA, SK * b + nt)
        eng.wait_ge(s_cpD, SK * b + nt)
        out_ap, in_ap = store_batch_aps(b)
        eng.dma_start(out=out_ap, in_=in_ap).then_inc(sob[b % OG], 16)

    def batches_per_slot(o):
        return len(range(o, N_SB, OG))

    with nc.Block() as block:

        # SP ring: all x loads (LK-tile groups) and all y stores (SK-tile
        # batches). Store batch b is emitted once 2b+1 <= 4g-10, i.e. ~12
        # tiles behind the load head, so ring-order readiness stays
        # monotone (a tighter interleave would head-of-line block the
        # loads and collapse the prefetch pipeline).
        @block.sync
        def _(sync):
            b_next = 0
            for g in range(N_LG):
                if g >= XG:
                    # xb group slot reuse: group g-XG fully consumed by PE
                    sync.wait_ge(s_mm, N_CHUNKS * LK * (g - XG + 1))
                out_ap, in_ap = load_group_aps(g)
                sync.dma_start(out=out_ap, in_=in_ap).then_inc(
                    sxg[g % XG], 16)
                while b_next < N_SB and SK * b_next + 1 <= LK * g - 10:
                    emit_store(sync, b_next)
                    b_next += 1
            while b_next < N_SB:
                emit_store(sync, b_next)
                b_next += 1
            for o in range(OG):
                sync.wait_ge(sob[o], 16 * batches_per_slot(o))

        @block.tensor
        def _(tensor):
            tensor.wait_ge(sm, 16)
            for t in range(N_TILES):
                g = t // LK
                if t % LK == 0:
                    tensor.wait_ge(sxg[g % XG], 16 * (g // XG + 1))
                xcb = ((g % XG) * LK + (t % LK)) * IN_COLS
                for ci in range(N_CHUNKS):
                    gc = t * N_CHUNKS + ci
                    if gc >= 8:
                        # PSUM bank free once chunk gc-8's copy retired;
                        # chunk parity == bank parity (8 banks, 2 chunks)
                        if ci == 0:
                            tensor.wait_ge(s_cpA, t - 3)
                        else:
                            tensor.wait_ge(s_cpD, t - 3)
                    c0 = ci * CHUNK
                    for dx in range(KW):
                        ins = nc.tensor.matmul(
                            pb[gc % 8][:TILE_OUT, :CHUNK],
                            mt[:TILE_IN, dx * TILE_OUT:(dx + 1) * TILE_OUT],
                            xb[:TILE_IN, xcb + c0 + dx:xcb + c0 + dx + CHUNK],
                            start=(dx == 0),
                            stop=(dx == KW - 1),
                        )
                        if dx == KW - 1:
                            ins.then_inc(s_mm, 1)

        @block.scalar
        def _(scalar):
            scalar.dma_start(out=mt, in_=m[:]).then_inc(sm, 16)
            for t in range(N_TILES):
                bt = t // SK
                if bt >= OG:
                    # ob batch slot reuse: batch bt-OG's store retired
                    scalar.wait_ge(sob[bt % OG], 16 * (bt // OG))
                scalar.wait_ge(s_mm, N_CHUNKS * t + 1)
                oc = (t % OBUFS) * COLS_PER_CORE
                nc.scalar.copy(
                    out=ob[:TILE_OUT, oc:oc + CHUNK],
                    in_=pb[(N_CHUNKS * t) % 8][:TILE_OUT, :CHUNK],
                ).then_inc(s_cpA, 1)

        @block.vector
        def _(vector):
            for t in range(N_TILES):
                bt = t // SK
                if bt >= OG:
                    vector.wait_ge(sob[bt % OG], 16 * (bt // OG))
                vector.wait_ge(s_mm, N_CHUNKS * t + 2)
                oc = (t % OBUFS) * COLS_PER_CORE
                nc.vector.tensor_copy(
                    out=ob[:TILE_OUT, oc + CHUNK:oc + COLS_PER_CORE],
                    in_=pb[(N_CHUNKS * t + 1) % 8][:TILE_OUT, :CHUNK],
                ).then_inc(s_cpD, 1)

    return nc


def _get_program():
    if "nc" not in _NC_CACHE:
        _NC_CACHE["nc"] = _build_program()
    return _NC_CACHE["nc"]


def _band_matrices(weight: np.ndarray) -> np.ndarray:
    """m[k, dx*126 + y] = w[k-y, dx] for 0 <= k-y < 3."""
    mm = np.zeros((128, KW * TILE_OUT), dtype=np.float32)
    for dx in range(KW):
        for dy in range(KH):
            ys = np.arange(TILE_OUT)
            mm[ys + dy, dx * TILE_OUT + ys] = weight[dy, dx]
    return mm.astype(np.float16)


def _in_maps(x, weight):
    mmat = _band_matrices(weight)
    xh = np.asarray(x, dtype=np.float32).astype(np.float16)
    maps = []
    for i in range(N_CORES):
        c0 = i * COLS_PER_CORE
        c1 = min(c0 + IN_COLS, W)
        slab = np.zeros((H, IN_COLS), dtype=np.float16)
        slab[:, : c1 - c0] = xh[:, c0:c1]
        maps.append({"x": np.ascontiguousarray(slab), "m": mmat})
    return maps


def kernel(x: np.ndarray, weight: np.ndarray) -> np.ndarray:
    x = np.asarray(x, dtype=np.float32)
    weight = np.asarray(weight, dtype=np.float32)
    assert x.shape == (H, W) and weight.shape == (KH, KW)

    nc = _get_program()
    res = run_bass_kernel_spmd(nc, _in_maps(x, weight),
                               core_ids=list(range(N_CORES)))

    out = np.empty((OUT_H, OUT_W), dtype=np.float32)
    for i in range(N_CORES):
        c0 = i * COLS_PER_CORE
        keep = min(COLS_PER_CORE, OUT_W - c0)
        out[:, c0:c0 + keep] = res.results[i]["y"][:, :keep].astype(np.float32)
    return out


# revision 13
# speedup vs baseline: 2.6917x; 1.0407x over previous
"""Trainium2 Bass kernel: 3x3 valid 2D cross-correlation on an 8192x8192 f32 image.

Strategy (8 NeuronCores, pure spatial/data parallel, fp16 I/O):
  - The rel-err budget (2e-2) admits fp16 end-to-end (~3.5e-4 actual), which
    halves HBM traffic vs f32: per core ~17.1 MB read + ~16.8 MB write at
    ~400 GB/s/core => ~85 us DMA floor (the kernel is HBM-bandwidth-bound,
    with the TensorEngine nearly co-critical at ~92 us).
  - Column-shard on the host: core i receives the fp16 input column slab
    [8192, 1026] = cols [1024*i, 1024*i + 1026) (2-col halo; right edge
    zero-padded, garbage output cols discarded on host).
  - Per core, 65 identical row tiles: [128 in rows -> 126 out rows] x 1024
    out cols (65*126 = 8190 exactly; tile t reads rows [126t, 126t+128)).
    Per tile, 2 PSUM chunks of 512 cols; per chunk 3 TensorEngine matmuls
    (fp16 operands, 1 col/cycle) accumulate:
        out[y, c] = sum_dx (M_dx.T @ X)[y, c+dx]
    where M_dx[k, y] = w[k-y, dx] is a 3-diagonal band matrix built on the
    host from the 3x3 weight.
  - DMA-instruction issue is expensive (~1.2 us seq+HWDGE per instruction),
    so transfers are batched via custom 3-D DRAM access patterns: loads move
    LK=4 tiles per instruction (the 2-row inter-tile halo makes the read AP
    overlap, which is legal), stores move SK=2 tiles. All x loads and all
    y stores issue on the SP ring; the m load goes on the ACT ring; ACT/DVE
    only do PSUM->SBUF copies (f32->fp16, even/odd chunks).
  - DMA completions can retire out of order across the 16 DMA engines, so
    every buffer slot gets its own semaphore (a single counting semaphore
    cannot prove a *specific* transfer finished).
"""

import numpy as np

import concourse.bass as bass
import concourse.mybir as mybir
from concourse.bass_utils import run_bass_kernel_spmd

H = W = 8192
KH = KW = 3
N_CORES = 8
OUT_H = H - KH + 1  # 8190
OUT_W = W - KW + 1  # 8190

COLS_PER_CORE = 1024          # output cols per core (core 7: keep 1022)
IN_COLS = COLS_PER_CORE + KW - 1  # 1026
TILE_OUT = 126                # output rows per 128-partition input tile
TILE_IN = TILE_OUT + KH - 1   # 128
N_TILES = OUT_H // TILE_OUT   # 65 (exact)
CHUNK = 512                   # PSUM bank width (fp32)
N_CHUNKS = COLS_PER_CORE // CHUNK  # 2

XBUFS = 16                    # xb tile slots (slot = t % 16)
XSEMS = 8                     # load-completion semaphores (round-robin)
# Load groups (t0, nt): small at the start so PE can begin ~2us in and the
# DMA power-ramp stragglers delay less work; steady-state 4-tile groups to
# amortize the ~1.2us per-instruction issue cost (seq + HWDGE).
LOADS = [(0, 1), (1, 1), (2, 2), (4, 2), (6, 2)] + [
    (t0, min(4, N_TILES - t0)) for t0 in range(8, N_TILES, 4)
]
SK = 2                        # tiles per store batch
OG = 8                        # ob batch slots (16 tiles of cushion)
OBUFS = OG * SK               # 16 tile slots
N_SB = (N_TILES - 1) // SK    # 32 full batches (tiles 0..63); tile 64 is
                              # stored as two 512-col half-stores at the end

_NC_CACHE = {}


def _build_program():
    nc = bass.Bass("TRN2", target_bir_lowering=False, debug=False)
    x = nc.declare_dram_parameter(
        "x", [H, IN_COLS], mybir.dt.float16, isOutput=False
    )
    m = nc.declare_dram_parameter(
        "m", [128, KW * TILE_OUT], mybir.dt.float16, isOutput=False
    )
    y = nc.declare_dram_parameter(
        "y", [OUT_H, COLS_PER_CORE], mybir.dt.float16, isOutput=True
    )

    xb = nc.alloc_sbuf_tensor(
        "xb", [128, XBUFS * IN_COLS], mybir.dt.float16).ap()
    ob = nc.alloc_sbuf_tensor(
        "ob", [128, OBUFS * COLS_PER_CORE], mybir.dt.float16).ap()
    mt = nc.alloc_sbuf_tensor("mt", [128, KW * TILE_OUT], mybir.dt.float16).ap()
    pb = [nc.alloc_psum_tensor(f"pb{i}", [128, CHUNK], mybir.dt.float32).ap()
          for i in range(8)]

    sm = nc.alloc_semaphore("sm")
    sxl = [nc.alloc_semaphore(f"sxl{s}") for s in range(XSEMS)]
    sob = [nc.alloc_semaphore(f"sob{o}") for o in range(OG)]
    s_mm = nc.alloc_semaphore("s_mm")
    s_cpA = nc.alloc_semaphore("s_cpA")
    s_cpD = nc.alloc_semaphore("s_cpD")

    # tile -> index of the load group that brings it in
    tile_load = {}
    for li, (t0, nt) in enumerate(LOADS):
        for t in range(t0, t0 + nt):
            tile_load[t] = li

    def load_group_aps(t0, nt):
        r0 = t0 * TILE_OUT
        in_ap = x[r0:r0 + TILE_IN, :].unsqueeze(1)
        in_ap.ap = mybir.VecI64Pair(
            [[IN_COLS, TILE_IN], [TILE_OUT * IN_COLS, nt], [1, IN_COLS]]
        )
        cb = (t0 % XBUFS) * IN_COLS
        out_ap = xb[:TILE_IN, cb:cb + nt * IN_COLS]
        return out_ap, in_ap

    def emit_store(eng, b):
        eng.wait_ge(s_cpA, SK * b + SK)
        eng.wait_ge(s_cpD, SK * b + SK)
        r0 = SK * b * TILE_OUT
        out_ap = y[r0:r0 + TILE_OUT, :].unsqueeze(1)
        out_ap.ap = mybir.VecI64Pair(
            [[COLS_PER_CORE, TILE_OUT],
             [TILE_OUT * COLS_PER_CORE, SK],
             [1, COLS_PER_CORE]]
        )
        cb = ((SK * b) % OBUFS) * COLS_PER_CORE
        in_ap = ob[:TILE_OUT, cb:cb + SK * COLS_PER_CORE]
        eng.dma_start(out=out_ap, in_=in_ap).then_inc(sob[b % OG], 16)

    with nc.Block() as block:

        # SP ring: all x loads and all y stores. Store batch b is emitted
        # once its tiles are >= 14 behind the load head, so ring-order
        # readiness stays monotone (a tighter interleave would head-of-line
        # block the loads and collapse the prefetch pipeline). Tile 64 is
        # stored as two 512-col halves, each released by its own copy
        # engine, to shorten the drain.
        @block.sync
        def _(sync):
            b_next = 0
            for li, (t0, nt) in enumerate(LOADS):
                if t0 >= XBUFS:
                    # xb slot reuse: previous occupants fully consumed
                    sync.wait_ge(s_mm, N_CHUNKS * (t0 - XBUFS + nt))
                out_ap, in_ap = load_group_aps(t0, nt)
                sync.dma_start(out=out_ap, in_=in_ap).then_inc(
                    sxl[li % XSEMS], 16)
                while b_next < N_SB and SK * b_next + 1 <= t0 + nt - 14:
                    emit_store(sync, b_next)
                    b_next += 1
            while b_next < N_SB:
                emit_store(sync, b_next)
                b_next += 1
            # tile 64 half-stores
            tl = N_TILES - 1
            r0 = tl * TILE_OUT
            oc = (tl % OBUFS) * COLS_PER_CORE
            sync.wait_ge(s_cpA, tl + 1)
            sync.dma_start(
                out=y[r0:r0 + TILE_OUT, 0:CHUNK],
                in_=ob[:TILE_OUT, oc:oc + CHUNK],
            ).then_inc(sob[0], 16)
            sync.wait_ge(s_cpD, tl + 1)
            sync.dma_start(
                out=y[r0:r0 + TILE_OUT, CHUNK:COLS_PER_CORE],
                in_=ob[:TILE_OUT, oc + CHUNK:oc + COLS_PER_CORE],
            ).then_inc(sob[0], 16)
            for o in range(OG):
                n = len(range(o, N_SB, OG)) * 16 + (32 if o == 0 else 0)
                sync.wait_ge(sob[o], n)

        @block.tensor
        def _(tensor):
            tensor.wait_ge(sm, 16)
            for t in range(N_TILES):
                li = tile_load[t]
                if t == LOADS[li][0]:
                    tensor.wait_ge(sxl[li % XSEMS], 16 * (li // XSEMS + 1))
                xcb = (t % XBUFS) * IN_COLS
                for ci in range(N_CHUNKS):
                    gc = t * N_CHUNKS + ci
                    if gc >= 8:
                        # PSUM bank free once chunk gc-8's copy retired;
                        # chunk parity == bank parity (8 banks, 2 chunks)
                        if ci == 0:
                            tensor.wait_ge(s_cpA, t - 3)
                        else:
                            tensor.wait_ge(s_cpD, t - 3)
                    c0 = ci * CHUNK
                    for dx in range(KW):
                        ins = nc.tensor.matmul(
                            pb[gc % 8][:TILE_OUT, :CHUNK],
                            mt[:TILE_IN, dx * TILE_OUT:(dx + 1) * TILE_OUT],
                            xb[:TILE_IN, xcb + c0 + dx:xcb + c0 + dx + CHUNK],
                            start=(dx == 0),
                            stop=(dx == KW - 1),
                        )
                        if dx == KW - 1:
                            ins.then_inc(s_mm, 1)

        @block.scalar
        def _(scalar):
            scalar.dma_start(out=mt, in_=m[:]).then_inc(sm, 16)
            for t in range(N_TILES):
                bt = t // SK
                if bt >= OG:
                    # ob batch slot reuse: batch bt-OG's store retired
                    scalar.wait_ge(sob[bt % OG], 16 * (bt // OG))
                scalar.wait_ge(s_mm, N_CHUNKS * t + 1)
                oc = (t % OBUFS) * COLS_PER_CORE
                nc.scalar.copy(
                    out=ob[:TILE_OUT, oc:oc + CHUNK],
                    in_=pb[(N_CHUNKS * t) % 8][:TILE_OUT, :CHUNK],
                ).then_inc(s_cpA, 1)

        @block.vector
        def _(vector):
            for t in range(N_TILES):
                bt = t // SK
                if bt >= OG:
                    vector.wait_ge(sob[bt % OG], 16 * (bt // OG))
                vector.wait_ge(s_mm, N_CHUNKS * t + 2)
                oc = (t % OBUFS) * COLS_PER_CORE
                nc.vector.tensor_copy(
                    out=ob[:TILE_OUT, oc + CHUNK:oc + COLS_PER_CORE],
                    in_=pb[(N_CHUNKS * t + 1) % 8][:TILE_OUT, :CHUNK],
                ).then_inc(s_cpD, 1)

    return nc


def _get_program():
    if "nc" not in _NC_CACHE:
        _NC_CACHE["nc"] = _build_program()
    return _NC_CACHE["nc"]


def _band_matrices(weight: np.ndarray) -> np.ndarray:
    """m[k, dx*126 + y] = w[k-y, dx] for 0 <= k-y < 3."""
    mm = np.zeros((128, KW * TILE_OUT), dtype=np.float32)
    for dx in range(KW):
        for dy in range(KH):
            ys = np.arange(TILE_OUT)
            mm[ys + dy, dx * TILE_OUT + ys] = weight[dy, dx]
    return mm.astype(np.float16)


def _in_maps(x, weight):
    mmat = _band_matrices(weight)
    xh = np.asarray(x, dtype=np.float32).astype(np.float16)
    maps = []
    for i in range(N_CORES):
        c0 = i * COLS_PER_CORE
        c1 = min(c0 + IN_COLS, W)
        slab = np.zeros((H, IN_COLS), dtype=np.float16)
        slab[:, : c1 - c0] = xh[:, c0:c1]
        maps.append({"x": np.ascontiguousarray(slab), "m": mmat})
    return maps


def kernel(x: np.ndarray, weight: np.ndarray) -> np.ndarray:
    x = np.asarray(x, dtype=np.float32)
    weight = np.asarray(weight, dtype=np.float32)
    assert x.shape == (H, W) and weight.shape == (KH, KW)

    nc = _get_program()
    res = run_bass_kernel_spmd(nc, _in_maps(x, weight),
                               core_ids=list(range(N_CORES)))

    out = np.empty((OUT_H, OUT_W), dtype=np.float32)
    for i in range(N_CORES):
        c0 = i * COLS_PER_CORE
        keep = min(COLS_PER_CORE, OUT_W - c0)
        out[:, c0:c0 + keep] = res.results[i]["y"][:, :keep].astype(np.float32)
    return out


# revision 16
# speedup vs baseline: 2.7413x; 1.0184x over previous
"""Trainium2 Bass kernel: 3x3 valid 2D cross-correlation on an 8192x8192 f32 image.

Strategy (8 NeuronCores, pure spatial/data parallel, fp16 I/O):
  - The rel-err budget (2e-2) admits fp16 end-to-end (~3.5e-4 actual), which
    halves HBM traffic vs f32: per core ~17.1 MB read + ~16.8 MB write at
    ~400 GB/s/core => ~85 us DMA floor (the kernel is HBM-bandwidth-bound,
    with the TensorEngine nearly co-critical at ~92 us).
  - Column-shard on the host: core i receives the fp16 input column slab
    [8192, 1026] = cols [1024*i, 1024*i + 1026) (2-col halo; right edge
    zero-padded, garbage output cols discarded on host).
  - Per core, 65 identical row tiles: [128 in rows -> 126 out rows] x 1024
    out cols (65*126 = 8190 exactly; tile t reads rows [126t, 126t+128)).
    Per tile, 2 PSUM chunks of 512 cols; per chunk 3 TensorEngine matmuls
    (fp16 operands, 1 col/cycle) accumulate:
        out[y, c] = sum_dx (M_dx.T @ X)[y, c+dx]
    where M_dx[k, y] = w[k-y, dx] is a 3-diagonal band matrix built on the
    host from the 3x3 weight.
  - DMA-instruction issue is expensive (~1.2 us seq+HWDGE per instruction),
    so transfers are batched via custom 3-D DRAM access patterns: loads move
    LK=4 tiles per instruction (the 2-row inter-tile halo makes the read AP
    overlap, which is legal), stores move SK=2 tiles. All x loads and all
    y stores issue on the SP ring; the m load goes on the ACT ring; ACT/DVE
    only do PSUM->SBUF copies (f32->fp16, even/odd chunks).
  - DMA completions can retire out of order across the 16 DMA engines, so
    every buffer slot gets its own semaphore (a single counting semaphore
    cannot prove a *specific* transfer finished).
"""

import numpy as np

import concourse.bass as bass
import concourse.mybir as mybir
from concourse.bass_utils import run_bass_kernel_spmd

H = W = 8192
KH = KW = 3
N_CORES = 8
OUT_H = H - KH + 1  # 8190
OUT_W = W - KW + 1  # 8190

COLS_PER_CORE = 1024          # output cols per core (core 7: keep 1022)
IN_COLS = COLS_PER_CORE + KW - 1  # 1026
TILE_OUT = 126                # output rows per 128-partition input tile
TILE_IN = TILE_OUT + KH - 1   # 128
N_TILES = OUT_H // TILE_OUT   # 65 (exact)
CHUNK = 512                   # PSUM bank width (fp32)
N_CHUNKS = COLS_PER_CORE // CHUNK  # 2

XBUFS = 16                    # xb tile slots (slot = t % 16)
XSEMS = 8                     # load-completion semaphores (round-robin)
# Load groups (t0, nt): small at the start so PE can begin ~2us in and the
# DMA power-ramp stragglers delay less work; steady-state 4-tile groups to
# amortize the ~1.2us per-instruction issue cost (seq + HWDGE).
LOADS = [(0, 1), (1, 1), (2, 2), (4, 2), (6, 2)] + [
    (t0, min(4, N_TILES - t0)) for t0 in range(8, N_TILES, 4)
]
SK = 2                        # tiles per store batch
OG = 8                        # ob batch slots (16 tiles of cushion)
OBUFS = OG * SK               # 16 tile slots
N_SB = (N_TILES - 1) // SK    # 32 full batches (tiles 0..63); tile 64 is
                              # stored whole from the ACT ring at the end

_NC_CACHE = {}


def _build_program():
    nc = bass.Bass("TRN2", target_bir_lowering=False, debug=False)
    x = nc.declare_dram_parameter(
        "x", [H, IN_COLS], mybir.dt.float16, isOutput=False
    )
    m = nc.declare_dram_parameter(
        "m", [128, KW * TILE_OUT], mybir.dt.float16, isOutput=False
    )
    y = nc.declare_dram_parameter(
        "y", [OUT_H, COLS_PER_CORE], mybir.dt.float16, isOutput=True
    )

    xb = nc.alloc_sbuf_tensor(
        "xb", [128, XBUFS * IN_COLS], mybir.dt.float16).ap()
    ob = nc.alloc_sbuf_tensor(
        "ob", [128, OBUFS * COLS_PER_CORE], mybir.dt.float16).ap()
    mt = nc.alloc_sbuf_tensor("mt", [128, KW * TILE_OUT], mybir.dt.float16).ap()
    pb = [nc.alloc_psum_tensor(f"pb{i}", [128, CHUNK], mybir.dt.float32).ap()
          for i in range(8)]

    sm = nc.alloc_semaphore("sm")
    sxl = [nc.alloc_semaphore(f"sxl{s}") for s in range(XSEMS)]
    sob = [nc.alloc_semaphore(f"sob{o}") for o in range(OG)]
    s_mm = nc.alloc_semaphore("s_mm")
    s_cpA = nc.alloc_semaphore("s_cpA")
    s_cpD = nc.alloc_semaphore("s_cpD")

    # tile -> index of the load group that brings it in
    tile_load = {}
    for li, (t0, nt) in enumerate(LOADS):
        for t in range(t0, t0 + nt):
            tile_load[t] = li

    def load_group_aps(t0, nt):
        r0 = t0 * TILE_OUT
        in_ap = x[r0:r0 + TILE_IN, :].unsqueeze(1)
        in_ap.ap = mybir.VecI64Pair(
            [[IN_COLS, TILE_IN], [TILE_OUT * IN_COLS, nt], [1, IN_COLS]]
        )
        cb = (t0 % XBUFS) * IN_COLS
        out_ap = xb[:TILE_IN, cb:cb + nt * IN_COLS]
        return out_ap, in_ap

    def emit_store(eng, b):
        eng.wait_ge(s_cpA, SK * b + SK)
        eng.wait_ge(s_cpD, SK * b + SK)
        r0 = SK * b * TILE_OUT
        out_ap = y[r0:r0 + TILE_OUT, :].unsqueeze(1)
        out_ap.ap = mybir.VecI64Pair(
            [[COLS_PER_CORE, TILE_OUT],
             [TILE_OUT * COLS_PER_CORE, SK],
             [1, COLS_PER_CORE]]
        )
        cb = ((SK * b) % OBUFS) * COLS_PER_CORE
        in_ap = ob[:TILE_OUT, cb:cb + SK * COLS_PER_CORE]
        eng.dma_start(out=out_ap, in_=in_ap).then_inc(sob[b % OG], 16)

    with nc.Block() as block:

        # SP ring: all x loads and all y stores except tile 64's. Store
        # batch b is emitted once its tiles are >= 14 behind the load head,
        # so ring-order readiness stays monotone (a tighter interleave
        # would head-of-line block the loads and collapse the prefetch
        # pipeline).
        @block.sync
        def _(sync):
            b_next = 0
            for li, (t0, nt) in enumerate(LOADS):
                if t0 >= XBUFS:
                    # xb slot reuse: previous occupants fully consumed
                    sync.wait_ge(s_mm, N_CHUNKS * (t0 - XBUFS + nt))
                out_ap, in_ap = load_group_aps(t0, nt)
                sync.dma_start(out=out_ap, in_=in_ap).then_inc(
                    sxl[li % XSEMS], 16)
                while b_next < N_SB and SK * b_next + 1 <= t0 + nt - 14:
                    emit_store(sync, b_next)
                    b_next += 1
            while b_next < N_SB:
                emit_store(sync, b_next)
                b_next += 1
            # tile 64 is stored whole on the ACT ring (see scalar block):
            # one DMA issue (~1.4 us engine hold each) in parallel with
            # SP's last batch beats serializing extra issues here
            for o in range(OG):
                n = len(range(o, N_SB, OG)) * 16 + (16 if o == 0 else 0)
                sync.wait_ge(sob[o], n)

        @block.tensor
        def _(tensor):
            tensor.wait_ge(sm, 16)
            for t in range(N_TILES):
                li = tile_load[t]
                if t == LOADS[li][0]:
                    tensor.wait_ge(sxl[li % XSEMS], 16 * (li // XSEMS + 1))
                xcb = (t % XBUFS) * IN_COLS
                for ci in range(N_CHUNKS):
                    gc = t * N_CHUNKS + ci
                    if gc >= 8:
                        # PSUM bank free once chunk gc-8's copy retired;
                        # chunk parity == bank parity (8 banks, 2 chunks)
                        if ci == 0:
                            tensor.wait_ge(s_cpA, t - 3)
                        else:
                            tensor.wait_ge(s_cpD, t - 3)
                    c0 = ci * CHUNK
                    for dx in range(KW):
                        ins = nc.tensor.matmul(
                            pb[gc % 8][:TILE_OUT, :CHUNK],
                            mt[:TILE_IN, dx * TILE_OUT:(dx + 1) * TILE_OUT],
                            xb[:TILE_IN, xcb + c0 + dx:xcb + c0 + dx + CHUNK],
                            start=(dx == 0),
                            stop=(dx == KW - 1),
                        )
                        if dx == KW - 1:
                            ins.then_inc(s_mm, 1)

        @block.scalar
        def _(scalar):
            scalar.dma_start(out=mt, in_=m[:]).then_inc(sm, 16)
            for t in range(N_TILES):
                bt = t // SK
                if bt >= OG:
                    # ob batch slot reuse: batch bt-OG's store retired
                    scalar.wait_ge(sob[bt % OG], 16 * (bt // OG))
                scalar.wait_ge(s_mm, N_CHUNKS * t + 1)
                oc = (t % OBUFS) * COLS_PER_CORE
                nc.scalar.copy(
                    out=ob[:TILE_OUT, oc:oc + CHUNK],
                    in_=pb[(N_CHUNKS * t) % 8][:TILE_OUT, :CHUNK],
                ).then_inc(s_cpA, 1)
            # store tile 64 whole from the ACT ring: chunk0's copy just
            # retired in-stream, only DVE's chunk1 copy needs a wait
            tl = N_TILES - 1
            r0 = tl * TILE_OUT
            oc = (tl % OBUFS) * COLS_PER_CORE
            scalar.wait_ge(s_cpD, tl + 1)
            scalar.dma_start(
                out=y[r0:r0 + TILE_OUT, :],
                in_=ob[:TILE_OUT, oc:oc + COLS_PER_CORE],
            ).then_inc(sob[0], 16)

        @block.vector
        def _(vector):
            for t in range(N_TILES):
                bt = t // SK
                if bt >= OG:
                    vector.wait_ge(sob[bt % OG], 16 * (bt // OG))
                vector.wait_ge(s_mm, N_CHUNKS * t + 2)
                oc = (t % OBUFS) * COLS_PER_CORE
                nc.vector.tensor_copy(
                    out=ob[:TILE_OUT, oc + CHUNK:oc + COLS_PER_CORE],
                    in_=pb[(N_CHUNKS * t + 1) % 8][:TILE_OUT, :CHUNK],
                ).then_inc(s_cpD, 1)

    return nc


def _get_program():
    if "nc" not in _NC_CACHE:
        _NC_CACHE["nc"] = _build_program()
    return _NC_CACHE["nc"]


def _band_matrices(weight: np.ndarray) -> np.ndarray:
    """m[k, dx*126 + y] = w[k-y, dx] for 0 <= k-y < 3."""
    mm = np.zeros((128, KW * TILE_OUT), dtype=np.float32)
    for dx in range(KW):
        for dy in range(KH):
            ys = np.arange(TILE_OUT)
            mm[ys + dy, dx * TILE_OUT + ys] = weight[dy, dx]
    return mm.astype(np.float16)


def _in_maps(x, weight):
    mmat = _band_matrices(weight)
    xh = np.asarray(x, dtype=np.float32).astype(np.float16)
    maps = []
    for i in range(N_CORES):
        c0 = i * COLS_PER_CORE
        c1 = min(c0 + IN_COLS, W)
        slab = np.zeros((H, IN_COLS), dtype=np.float16)
        slab[:, : c1 - c0] = xh[:, c0:c1]
        maps.append({"x": np.ascontiguousarray(slab), "m": mmat})
    return maps


def kernel(x: np.ndarray, weight: np.ndarray) -> np.ndarray:
    x = np.asarray(x, dtype=np.float32)
    weight = np.asarray(weight, dtype=np.float32)
    assert x.shape == (H, W) and weight.shape == (KH, KW)

    nc = _get_program()
    res = run_bass_kernel_spmd(nc, _in_maps(x, weight),
                               core_ids=list(range(N_CORES)))

    out = np.empty((OUT_H, OUT_W), dtype=np.float32)
    for i in range(N_CORES):
        c0 = i * COLS_PER_CORE
        keep = min(COLS_PER_CORE, OUT_W - c0)
        out[:, c0:c0 + keep] = res.results[i]["y"][:, :keep].astype(np.float32)
    return out
